# revision 14
# baseline (speedup 1.0000x reference)
"""TRN2 Bass kernel for nn_BaseAttention_46548855554192.

B=2, S=2048, H=2048, NH=16, HD=128 multi-head attention with RoPE and an
additive attention mask, computed tensor-parallel over heads on 8 NeuronCores
(2 heads per core).  Each core computes the qkv projection for its heads,
RoPE, causal softmax attention, and a partial o_proj (its head-columns of
o_w); the host sums the 8 partial outputs.

Layout strategy (per core):
  - hidden is fed transposed hT [H, B*S]; q,k computed as [feat, s] so the
    head dim (128) lands on SBUF partitions; v computed in [s, d] directly.
  - scores are computed transposed (scoresT [k, q]) so the softmax sum is a
    partition-dim reduction done on the PE (ones-matmul) and exp is a
    PSUM->SBUF ACT op; 1/sum comes from DVE reciprocal broadcast across
    partitions by the ones-matmul itself.
  - no max-subtraction in softmax (scores are O(10) for randn inputs; exp in
    f32 is safe to ~88; mathematically identical to the reference).
  - RoPE rotate_half is a permutation matmul (engines cannot move data
    across partitions); cos/sin tables are host-side (ACT Sin has no range
    reduction) with the rotation sign folded into the sin table.
  - causal masking: fully-masked k-tiles are skipped; the two diagonal
    k-tiles get an additive -1e5 staircase PRE-WRITTEN into PSUM (DVE copy,
    off the critical path) and the score matmuls accumulate on top, so no
    post-exp mask op sits between exp and the sum/PV matmuls.  The fully
    masked q-half of the second diagonal tile is skipped in scores/sum/PV.
  - causal mode interleaves qkv chunk n -> rope n -> attention q-block n so
    both batches stream without a serialization point.
  - everything streamed is bf16 (matmul rate at >=256 cols equals fp32r, but
    DMA bytes halve, DVE elementwise gets 2x, SBUF pressure halves); PSUM
    accumulation stays f32 except single-shot score/rope tiles (bf16).
"""

import numpy as np

import bass_rust
import concourse.bass as bass
import concourse.mybir as mybir
from concourse.tile import TileContext
from concourse.vector_clock import ScopedClock

F32 = mybir.dt.float32
BF16 = mybir.dt.bfloat16
AF = mybir.ActivationFunctionType
OP = mybir.AluOpType

B, S, H, NH, HD = 2, 2048, 2048, 16, 128
BS = B * S                  # 4096
HPC = NH // 8               # heads per core = 2
DLOC = HPC * HD             # local attn dims = 256
CH = 256                    # chunk / q-block width
NCH = S // CH               # 8 chunks per batch
KT = S // 128               # 16 k-tiles of 128 per batch
SCALE = 1.0 / float(np.sqrt(HD))
ROPE_BASE = 10000.0
MASK_NEG = -1.0e5           # additive mask; exp(SCALE*-1e5) == 0 in f32

LAG = 1
import os as _os
PS_SPLIT = _os.environ.get("K_PS_SPLIT", "3,2,3")  # ps,psO,psS (0=share ps)
OPROJ_DELAY = _os.environ.get("K_OPROJ_DELAY", "1") == "1"
SB_BUFS = 3
AP_BUFS = 6
OB_BUFS = 3
BIG_BUFS = 4
MAX_WAITS = 1  # this container's walrus supports one sync-wait per instruction


class PatchedTileContext(TileContext):
    """Split multi-sem waits into single-wait NOPs (old-walrus limitation)."""

    def _lower_ordered_insts(self, ordered):
        for bb_name, insts in ordered.items():
            new_list = []
            for inst in insts:
                si = inst.sync_info
                if si is not None and len(si.on_wait) > MAX_WAITS:
                    waits = list(si.on_wait)
                    keep = waits[:MAX_WAITS]
                    extra = waits[MAX_WAITS:]
                    scopes = self._inst_to_scopes.get(inst.name, ())
                    for i in range(0, len(extra), MAX_WAITS):
                        group = extra[i:i + MAX_WAITS]
                        nop = mybir.InstNoOp(
                            name=f"waitsplit-{self.nc.next_id()}",
                            engine=inst.engine,
                            sync_info=mybir.SyncInfo(on_wait=list(group), on_update=[]),
                            bass_nofuse=True,
                        )
                        self._inst_to_scopes[nop.name] = scopes
                        new_list.append(nop)
                    inst.sync_info = bass_rust.SyncInfo(
                        on_wait=keep, on_update=list(si.on_update)
                    )
                new_list.append(inst)
            insts[:] = new_list
        return super()._lower_ordered_insts(ordered)

    def _drain_and_barrier(self, tick_clock, wait_clock):
        nc = self.nc
        drain_inst = nc.sync.drain()
        wait_clock.add_sem_waits(
            drain_inst.ins, ScopedClock({None: tick_clock.global_clock})
        )
        si = drain_inst.ins.sync_info
        waits = list(si.on_wait) if si is not None else []
        if len(waits) > MAX_WAITS:
            assert self.sems is not None
            by_name = {h.name: h for h in self.sems.allocated().values()}
            keep = waits[:MAX_WAITS]
            extra = []
            for w in waits[MAX_WAITS:]:
                h = by_name.get(w.ant_name)
                if h is None:
                    keep.append(w)
                else:
                    extra.append((h, w.wait_value, w.wait_mode))
            drain_inst.ins.sync_info = bass_rust.SyncInfo(
                on_wait=keep, on_update=list(si.on_update) if si else []
            )
            for h, val, mode in extra:
                assert mode == "sem-ge-imm", mode
                nc.sync.wait_ge(h, val)

        nc.all_engine_barrier()
        assert self.sems is not None
        popped = nc._tile_sem_poison_stack.pop()
        assert popped is self._sem_poison
        nc.clear_and_free_semaphores(list(self.sems.allocated().values()))
        nc.all_engine_barrier()


def build_kernel(mask_mode: str) -> bass.Bass:
    """mask_mode: 'causal' (skip masked tiles), 'dense' (no mask),
    'generic' (additive mask streamed from DRAM)."""
    nc = bass.Bass()

    hT = nc.dram_tensor("hT", [H, BS], BF16, kind="ExternalInput")
    # m-major packed qk weights: [128, m(4) x kt(16) x 128]
    wqkP = nc.dram_tensor("wqkP", [128, 4 * KT * 128], BF16, kind="ExternalInput")
    wvP = nc.dram_tensor("wvP", [128, KT * DLOC], BF16, kind="ExternalInput")
    owP = nc.dram_tensor("owP", [128, 2 * H], BF16, kind="ExternalInput")
    bqkT = nc.dram_tensor("bqkT", [128, 4], F32, kind="ExternalInput")
    cosT = nc.dram_tensor("cosT", [128, BS], BF16, kind="ExternalInput")
    sinS = nc.dram_tensor("sinS", [128, BS], BF16, kind="ExternalInput")
    permP = nc.dram_tensor("permP", [128, 128], BF16, kind="ExternalInput")
    ones128 = nc.dram_tensor("ones128", [128, 128], BF16, kind="ExternalInput")
    if mask_mode == "causal":
        # additive staircase for the diagonal k-tile pair, [128, 512]:
        #   [0:256]   staircase for k-tile 2qb (0 where k<=q else -1e5)
        #   [256:384] all -1e5 (fully masked q-half of k-tile 2qb+1)
        #   [384:512] staircase for the live q-half of k-tile 2qb+1
        adiagD = nc.dram_tensor("adiag", [128, 512], BF16, kind="ExternalInput")
    if mask_mode == "generic":
        maskT = nc.dram_tensor("maskT", [B, S, S], F32, kind="ExternalInput")
    outP = nc.dram_tensor("outP", [BS, H], BF16, kind="ExternalOutput")

    causal = mask_mode == "causal"
    generic = mask_mode == "generic"

    with PatchedTileContext(nc) as tc:
        with (
            tc.tile_pool(name="const", bufs=1) as cpool,
            tc.tile_pool(name="work", bufs=2) as wpool,
            tc.tile_pool(name="sb", bufs=SB_BUFS) as sb,
            tc.tile_pool(name="bigp", bufs=BIG_BUFS) as bigp,
            tc.tile_pool(name="ap", bufs=AP_BUFS) as apool,
            tc.tile_pool(name="ob", bufs=OB_BUFS) as opool,
            tc.tile_pool(name="mp", bufs=8) as mp,
            # PSUM is bank-granular: 8 banks, split so slow-freed consumers
            # (o_proj evac, scores exp) don't gate unrelated allocations.
            tc.tile_pool(name="ps", bufs=int(PS_SPLIT.split(",")[0]), space="PSUM") as ps,
            tc.tile_pool(name="psO", bufs=max(1, int(PS_SPLIT.split(",")[1])), space="PSUM") as psO_,
            tc.tile_pool(name="psS", bufs=max(1, int(PS_SPLIT.split(",")[2])), space="PSUM") as psS_,
        ):
            # ---- resident constants ----
            # wqk is m-major so each m-column is one contiguous DMA; loads are
            # sliced and interleaved with the first h-chunk groups so the
            # first qkv matmuls start ~2us in.
            wqk_t = cpool.tile([128, 4 * KT * 128], BF16, tag="wqk")
            wv_t = cpool.tile([128, KT * DLOC], BF16, tag="wv")

            def load_wqk_m(m, half=None):
                if half is None:
                    lo, hi = m * 2048, (m + 1) * 2048
                else:
                    lo = m * 2048 + half * 1024
                    hi = lo + 1024
                nc.sync.dma_start(wqk_t[:, lo:hi], wqkP[:, lo:hi])

            def load_wv_half(h):
                lo, hi = h * (KT * DLOC // 2), (h + 1) * (KT * DLOC // 2)
                nc.sync.dma_start(wv_t[:, lo:hi], wvP[:, lo:hi])

            load_wqk_m(0, half=0)
            bqk_t = cpool.tile([128, 4], F32, tag="bqk")
            nc.sync.dma_start(bqk_t[:], bqkT[:, :])
            perm_t = cpool.tile([128, 128], BF16, tag="perm")
            nc.sync.dma_start(perm_t[:], permP[:, :])

            # late-loaded constants (first needed by attention q-block 0)
            ow_t = cpool.tile([128, 2 * H], BF16, tag="ow")
            ones_t = cpool.tile([128, 128], BF16, tag="ones")
            if causal:
                adiag_t = cpool.tile([128, 512], BF16, tag="adiag")

            def load_small_consts():
                nc.sync.dma_start(ones_t[:], ones128[:, :])
                if causal:
                    nc.sync.dma_start(adiag_t[:], adiagD[:, :])

            ow_loaded = [False]

            def load_ow():
                ow_loaded[0] = True
                nc.sync.dma_start(ow_t[:, 0:H], owP[:, 0:H])
                nc.sync.dma_start(ow_t[:, H:2 * H], owP[:, H:2 * H])

            for b in range(B):
                s_base = b * S
                qk_t = wpool.tile([128, 4 * S], BF16, tag="qkT")   # 4 m x [128,S]
                v_t = wpool.tile([128, KT * DLOC], BF16, tag="v")  # KT s-tiles
                cos_t = wpool.tile([128, S], BF16, tag="cos")
                sin_t = wpool.tile([128, S], BF16, tag="sin")

                def load_trig(half):
                    h0 = half * (S // 2)
                    nc.sync.dma_start(
                        cos_t[:, h0:h0 + S // 2],
                        cosT[:, s_base + h0:s_base + h0 + S // 2])
                    nc.sync.dma_start(
                        sin_t[:, h0:h0 + S // 2],
                        sinS[:, s_base + h0:s_base + h0 + S // 2])

                def load_chunk(n, interleave=()):
                    """Load h chunk n; optionally interleave other DMAs
                    between the kt-group DMAs (startup pipelining)."""
                    s0 = s_base + n * CH
                    h_t = bigp.tile([128, KT * CH], BF16, tag="big")
                    step = 4 if interleave else 8
                    il = list(interleave)
                    for ktg in range(0, KT, step):
                        nc.sync.dma_start(
                            h_t[:, ktg * CH:(ktg + step) * CH]
                            .rearrange("p (kt s) -> p kt s", kt=step),
                            hT[ktg * 128:(ktg + step) * 128, s0:s0 + CH]
                            .rearrange("(kt p) s -> p kt s", p=128),
                        )
                        if il:
                            il.pop(0)()
                    for fn in il:
                        fn()
                    return h_t

                def do_qkv_chunk(n, h_t):
                    if n + 1 < NCH:
                        # prefetch the next chunk so its DMA overlaps this
                        # chunk's compute and the sem fires well before use
                        h_tiles[n + 1] = load_chunk(n + 1)
                    if n == 0 and b == 0:
                        load_small_consts()
                        load_trig(0)
                    if n == 1 and not ow_loaded[0]:
                        load_ow()
                    if n == NCH // 2 - 1:
                        load_trig(1)
                    for m in range(4):  # q_h0,q_h1,k_h0,k_h1
                        p_qk = ps.tile([128, 512], F32, tag="ps")
                        for kt in range(KT):
                            nc.tensor.matmul(
                                p_qk[:, 0:256],
                                wqk_t[:, m * 2048 + kt * 128: m * 2048 + (kt + 1) * 128],
                                h_t[:, kt * CH:(kt + 1) * CH],
                                start=(kt == 0), stop=(kt == KT - 1),
                            )
                        nc.scalar.activation(
                            qk_t[:, m * S + n * CH: m * S + (n + 1) * CH],
                            p_qk[:, 0:256], AF.Identity, bias=bqk_t[:, m:m + 1],
                        )
                    for st in range(2):  # v in [s, d]
                        p_v = ps.tile([128, 512], F32, tag="ps")
                        for kt in range(KT):
                            nc.tensor.matmul(
                                p_v[:, 0:256],
                                h_t[:, kt * CH + st * 128: kt * CH + (st + 1) * 128],
                                wv_t[:, kt * DLOC:(kt + 1) * DLOC],
                                start=(kt == 0), stop=(kt == KT - 1),
                            )
                        stile = n * 2 + st
                        nc.scalar.activation(
                            v_t[:, stile * DLOC:(stile + 1) * DLOC], p_v[:, 0:256],
                            AF.Copy,
                        )

                def do_rope_chunk(n):
                    c0 = n * CH
                    for m in range(4):
                        qk_sl = qk_t[:, m * S + c0: m * S + c0 + CH]
                        p_rot = ps.tile([128, 512], F32, tag="ps")
                        nc.tensor.matmul(
                            p_rot[:, 0:256], perm_t[:], qk_sl, start=True, stop=True
                        )
                        rot_sb = sb.tile([128, CH], BF16, tag="rot")
                        # fused evac: rot_sb = psum_rot * sinS  (DVE reads PSUM)
                        nc.vector.tensor_tensor(
                            rot_sb[:], p_rot[:, 0:256], sin_t[:, c0:c0 + CH], OP.mult
                        )
                        t2 = sb.tile([128, CH], BF16, tag="t2")
                        nc.vector.tensor_tensor(
                            t2[:], qk_sl, cos_t[:, c0:c0 + CH], OP.mult
                        )
                        nc.vector.tensor_tensor(qk_sl, t2[:], rot_sb[:], OP.add)

                def attention_stage_a(qb):
                    """scores -> exp -> sum -> PV -> 1/sum -> at tiles.
                    Both heads' scores+exp are emitted first so each head's
                    exp tail drains under the other head's PE work."""
                    n_kt = 2 * (qb + 1) if causal else KT
                    if generic:
                        mask_tiles = []
                        for pair in range(n_kt // 2):
                            mt = mp.tile([128, 512], F32, tag="mask")
                            nc.sync.dma_start(
                                mt[:].rearrange("p (t q) -> p t q", t=2),
                                maskT[b, pair * 256:(pair + 1) * 256,
                                      qb * CH:(qb + 1) * CH]
                                .rearrange("(t p) q -> p t q", p=128),
                            )
                            mask_tiles.append(mt)
                    ex_tiles = []
                    for hh in range(HPC):
                        qof = hh * S
                        kof = (2 + hh) * S
                        q_full = qk_t[:, qof + qb * CH: qof + (qb + 1) * CH]
                        ex_t = bigp.tile([128, KT * CH], BF16, tag="big")
                        for pair in range((n_kt + 1) // 2):
                            kt0 = 2 * pair
                            diag = causal and (pair == qb)
                            p_sc = (psS_ if int(PS_SPLIT.split(",")[2]) else ps).tile([128, 512], F32, tag="ps2")
                            if diag:
                                # pre-write additive staircase; score matmuls
                                # accumulate on top (start=False; Pool engine,
                                # off the exp->PV critical path)
                                nc.gpsimd.tensor_copy(p_sc[:], adiag_t[:])
                            for gi in range(2):
                                kt = kt0 + gi
                                if kt >= n_kt:
                                    break
                                k_sl = qk_t[:, kof + kt * 128: kof + (kt + 1) * 128]
                                if diag and gi == 1:
                                    # only the live q-half of the 2nd diag tile
                                    nc.tensor.matmul(
                                        p_sc[:, 384:512], k_sl,
                                        qk_t[:, qof + qb * CH + 128:
                                             qof + (qb + 1) * CH],
                                        start=False, stop=True,
                                        skip_group_check=True,
                                    )
                                else:
                                    nc.tensor.matmul(
                                        p_sc[:, gi * CH:(gi + 1) * CH],
                                        k_sl, q_full,
                                        start=not diag, stop=True,
                                        skip_group_check=True,
                                    )
                                if generic:
                                    mt = mask_tiles[pair]
                                    nc.vector.tensor_tensor(
                                        p_sc[:, gi * CH:(gi + 1) * CH],
                                        p_sc[:, gi * CH:(gi + 1) * CH],
                                        mt[:, gi * CH:(gi + 1) * CH], OP.add,
                                    )
                            w = min(2, n_kt - kt0)
                            nc.scalar.activation(
                                ex_t[:, kt0 * CH:(kt0 + w) * CH],
                                p_sc[:, 0:w * CH], AF.Exp, scale=SCALE,
                            )
                        ex_tiles.append(ex_t)
                    at_tiles = []
                    for hh in range(HPC):
                        ex_t = ex_tiles[hh]
                        # softmax denominator: ones-matmul partition reduction
                        p_sum = ps.tile([128, 512], F32, tag="ps")
                        for kt in range(n_kt):
                            half = causal and (kt == 2 * qb + 1)
                            nc.tensor.matmul(
                                p_sum[:, 128:256] if half else p_sum[:, 0:256],
                                ones_t[:],
                                ex_t[:, kt * CH + (128 if half else 0):
                                     (kt + 1) * CH],
                                start=(kt == 0), stop=(kt == n_kt - 1),
                                skip_group_check=True,
                            )
                        # PV
                        p_at = ps.tile([128, 512], F32, tag="ps")
                        for kt in range(n_kt):
                            half = causal and (kt == 2 * qb + 1)
                            nc.tensor.matmul(
                                p_at[:, 128:256] if half else p_at[:, 0:256],
                                v_t[:, kt * DLOC + hh * 128: kt * DLOC + (hh + 1) * 128],
                                ex_t[:, kt * CH + (128 if half else 0):
                                     (kt + 1) * CH],
                                start=(kt == 0), stop=(kt == n_kt - 1),
                                skip_group_check=True,
                            )
                        # 1/sum (all 128 partition rows of p_sum are identical)
                        rec_t = sb.tile([128, CH], F32, tag="rec")
                        nc.vector.reciprocal(rec_t[:], p_sum[:, 0:256])
                        at_t = apool.tile([128, CH], BF16, tag="attn")
                        nc.vector.tensor_tensor(at_t[:], p_at[:, 0:256], rec_t[:], OP.mult)
                        at_tiles.append(at_t)
                    return at_tiles

                def do_oproj(qb, at_tiles):
                    # o_proj for this q-block; stage two 512-e chunks per
                    # [128,1024] tile so output DMAs stay large
                    last_block = (b == B - 1) and (qb == NCH - 1)
                    for ss in range(CH // 128):
                        for eg in range(H // 1024):
                            # the big pool's slots are idle at the kernel tail;
                            # borrow them so the final stores pipeline deeper
                            if last_block:
                                o_sb = bigp.tile([128, 1024], BF16, tag="big")
                            else:
                                o_sb = opool.tile([128, 1024], BF16, tag="osb")
                            for sub in range(2):
                                ec = eg * 2 + sub
                                p_o = (psO_ if int(PS_SPLIT.split(",")[1]) else ps).tile([128, 512], F32, tag="po")
                                for hh in range(HPC):
                                    nc.tensor.matmul(
                                        p_o[:],
                                        at_tiles[hh][:, ss * 128:(ss + 1) * 128],
                                        ow_t[:, hh * H + ec * 512: hh * H + (ec + 1) * 512],
                                        start=(hh == 0), stop=(hh == HPC - 1),
                                    )
                                dst = o_sb[:, sub * 512:(sub + 1) * 512]
                                if (ss * 2 + eg) % 2 == 0:
                                    nc.vector.tensor_copy(dst, p_o[:])
                                else:
                                    nc.gpsimd.tensor_copy(dst, p_o[:])
                            nc.sync.dma_start(
                                outP[s_base + qb * CH + ss * 128:
                                     s_base + qb * CH + (ss + 1) * 128,
                                     eg * 1024:(eg + 1) * 1024],
                                o_sb[:],
                            )

                h_tiles = {}
                if b == 0:
                    h_tiles[0] = load_chunk(0, interleave=[
                        lambda: load_wqk_m(0, half=1),
                        lambda: load_wqk_m(1),
                        lambda: (load_wqk_m(2), load_wqk_m(3)),
                        lambda: (load_wv_half(0), load_wv_half(1)),
                    ])
                else:
                    h_tiles[0] = load_chunk(0)
                if causal:
                    # software pipeline: attention stage A (scores..at) for
                    # q-block n-1 is emitted with chunk n; its o_proj is
                    # delayed one more slot so the at-mult chain drains under
                    # chunk n+1's qkv matmuls.
                    pend = {}
                    for n in range(NCH):
                        do_qkv_chunk(n, h_tiles.pop(n))
                        do_rope_chunk(n)
                        if OPROJ_DELAY:
                            if n - 1 - LAG >= 0:
                                do_oproj(n - 1 - LAG, pend.pop(n - 1 - LAG))
                            if n >= LAG:
                                pend[n - LAG] = attention_stage_a(n - LAG)
                        elif n >= LAG:
                            do_oproj(n - LAG, attention_stage_a(n - LAG))
                    if OPROJ_DELAY:
                        for qb in range(NCH - LAG - 1, NCH):
                            if qb in pend:
                                do_oproj(qb, pend.pop(qb))
                            if qb + LAG < NCH:
                                pend[qb + LAG] = attention_stage_a(qb + LAG)
                    else:
                        for t in range(LAG):
                            do_oproj(NCH - LAG + t,
                                     attention_stage_a(NCH - LAG + t))
                else:
                    for n in range(NCH):
                        do_qkv_chunk(n, h_tiles.pop(n))
                    for n in range(NCH):
                        do_rope_chunk(n)
                    for qb in range(NCH):
                        do_oproj(qb, attention_stage_a(qb))
    return nc


def _adiag_pattern():
    p = np.arange(128)[:, None]
    j = np.arange(512)[None, :]
    out = np.zeros((128, 512), dtype=np.float32)
    out[:, 0:256] = np.where(p <= j[:, 0:256], 0.0, MASK_NEG)
    out[:, 256:384] = MASK_NEG
    out[:, 384:512] = np.where(p <= (j[:, 384:512] - 384), 0.0, MASK_NEG)
    return out


def _host_prep(hidden_states, position_ids, attention_mask, qkv_w, qkv_b, o_w):
    import ml_dtypes
    bf16 = ml_dtypes.bfloat16

    hidden_states = np.asarray(hidden_states, dtype=np.float32)
    position_ids = np.asarray(position_ids)
    attention_mask = np.asarray(attention_mask, dtype=np.float32)
    qkv_w = np.asarray(qkv_w, dtype=np.float32)
    qkv_b = np.asarray(qkv_b, dtype=np.float32)
    o_w = np.asarray(o_w, dtype=np.float32)

    # mask mode detection
    causal = np.triu(np.full((S, S), -1e9, dtype=np.float32), k=1)
    m = attention_mask.reshape(B, S, S)
    if all(np.array_equal(m[b], causal) for b in range(B)):
        mask_mode = "causal"
    elif not attention_mask.any():
        mask_mode = "dense"
    else:
        mask_mode = "generic"

    # rope tables
    half = HD // 2
    inv = (1.0 / ROPE_BASE ** (np.arange(half, dtype=np.float64) / half))
    freqs = position_ids.astype(np.float64).reshape(BS, 1) * inv[None, :]  # [BS,64]
    c = np.cos(freqs).T  # [64, BS]
    s_ = np.sin(freqs).T
    cosT = np.concatenate([c, c], 0).astype(bf16)
    sinS = np.concatenate([-s_, s_], 0).astype(bf16)

    hT = np.ascontiguousarray(hidden_states.reshape(BS, H).T).astype(bf16)

    perm = np.zeros((128, 128), dtype=np.float32)
    for dp in range(128):
        perm[(dp + 64) % 128, dp] = 1.0  # out[dp] = in[(dp+64)%128]

    shared = {
        "hT": hT, "cosT": cosT, "sinS": sinS,
        "permP": perm.astype(bf16),
        "ones128": np.ones((128, 128), dtype=bf16),
    }
    if mask_mode == "causal":
        shared["adiag"] = _adiag_pattern().astype(bf16)
    if mask_mode == "generic":
        shared["maskT"] = np.ascontiguousarray(
            np.transpose(m, (0, 2, 1)) / SCALE
        ).astype(np.float32)

    in_maps = []
    for c_id in range(8):
        r = c_id * DLOC
        wqk = np.vstack([qkv_w[r:r + DLOC], qkv_w[H + r:H + r + DLOC]])      # [512, H]
        wv = qkv_w[2 * H + r: 2 * H + r + DLOC]                               # [256, H]
        bqk = np.concatenate([qkv_b[r:r + DLOC], qkv_b[H + r:H + r + DLOC]])  # [512]
        im = dict(shared)
        # m-major packing: wqkP[p, m*KT*128 + kt*128 + c] = wqk[m*128+c, kt*128+p]
        wqkT = np.ascontiguousarray(wqk.T)                 # [H, 512]
        im["wqkP"] = np.ascontiguousarray(
            wqkT.reshape(KT, 128, 4, 128).transpose(1, 2, 0, 3).reshape(128, -1)
        ).astype(bf16)
        wvT = np.ascontiguousarray(wv.T)                   # [H, 256]
        im["wvP"] = np.ascontiguousarray(
            wvT.reshape(KT, 128, DLOC).transpose(1, 0, 2).reshape(128, -1)
        ).astype(bf16)
        owT = np.ascontiguousarray(o_w[:, r:r + DLOC].T)   # [256, H]
        im["owP"] = np.ascontiguousarray(
            owT.reshape(2, 128, H).transpose(1, 0, 2).reshape(128, -1)
        ).astype(bf16)
        im["bqkT"] = np.ascontiguousarray(bqk.reshape(4, 128).T)
        in_maps.append(im)
    post_bias = qkv_b[2 * H:3 * H] @ o_w.T  # [H], exact since sum(probs)=1
    return mask_mode, in_maps, post_bias


def kernel(**inputs) -> np.ndarray:
    import os
    import sys
    # The devices are reached through the axon PJRT proxy; make sure a
    # JAX_PLATFORMS=cpu pin (used for CPU-side reference runs) doesn't hide
    # them if jax hasn't been imported yet.
    if os.environ.get("JAX_PLATFORMS") == "cpu" and "jax" not in sys.modules:
        del os.environ["JAX_PLATFORMS"]
    from concourse.bass_utils import run_bass_kernel_spmd

    mask_mode, in_maps, post_bias = _host_prep(**inputs)
    nc = build_kernel(mask_mode)
    res = run_bass_kernel_spmd(nc, in_maps, core_ids=list(range(8)), trace=False)
    out = np.zeros((BS, H), dtype=np.float64)
    for r in res.results:
        out += r["outP"].astype(np.float64)
    out += post_bias.astype(np.float64)[None, :]
    return out.astype(np.float32).reshape(B, S, H)


# revision 15
# speedup vs baseline: 1.0189x; 1.0189x over previous
"""TRN2 Bass kernel for nn_BaseAttention_46548855554192.

B=2, S=2048, H=2048, NH=16, HD=128 multi-head attention with RoPE and an
additive attention mask, computed tensor-parallel over heads on 8 NeuronCores
(2 heads per core).  Each core computes the qkv projection for its heads,
RoPE, causal softmax attention, and a partial o_proj (its head-columns of
o_w); the host sums the 8 partial outputs.

Layout strategy (per core):
  - hidden is fed transposed hT [H, B*S]; q,k computed as [feat, s] so the
    head dim (128) lands on SBUF partitions; v computed in [s, d] directly.
  - scores are computed transposed (scoresT [k, q]) so the softmax sum is a
    partition-dim reduction done on the PE (ones-matmul) and exp is a
    PSUM->SBUF ACT op; 1/sum comes from DVE reciprocal broadcast across
    partitions by the ones-matmul itself.
  - no max-subtraction in softmax (scores are O(10) for randn inputs; exp in
    f32 is safe to ~88; mathematically identical to the reference).
  - RoPE rotate_half is a permutation matmul (engines cannot move data
    across partitions); cos/sin tables are host-side (ACT Sin has no range
    reduction) with the rotation sign folded into the sin table.
  - causal masking: fully-masked k-tiles are skipped; the two diagonal
    k-tiles get an additive -1e5 staircase PRE-WRITTEN into PSUM (DVE copy,
    off the critical path) and the score matmuls accumulate on top, so no
    post-exp mask op sits between exp and the sum/PV matmuls.  The fully
    masked q-half of the second diagonal tile is skipped in scores/sum/PV.
  - causal mode interleaves qkv chunk n -> rope n -> attention q-block n so
    both batches stream without a serialization point.
  - everything streamed is bf16 (matmul rate at >=256 cols equals fp32r, but
    DMA bytes halve, DVE elementwise gets 2x, SBUF pressure halves); PSUM
    accumulation stays f32 except single-shot score/rope tiles (bf16).
"""

import numpy as np

import bass_rust
import concourse.bass as bass
import concourse.mybir as mybir
from concourse.tile import TileContext
from concourse.vector_clock import ScopedClock

F32 = mybir.dt.float32
BF16 = mybir.dt.bfloat16
AF = mybir.ActivationFunctionType
OP = mybir.AluOpType

B, S, H, NH, HD = 2, 2048, 2048, 16, 128
BS = B * S                  # 4096
HPC = NH // 8               # heads per core = 2
DLOC = HPC * HD             # local attn dims = 256
CH = 256                    # chunk / q-block width
NCH = S // CH               # 8 chunks per batch
KT = S // 128               # 16 k-tiles of 128 per batch
SCALE = 1.0 / float(np.sqrt(HD))
ROPE_BASE = 10000.0
MASK_NEG = -1.0e5           # additive mask; exp(SCALE*-1e5) == 0 in f32

LAG = 1
import os as _os
PS_SPLIT = _os.environ.get("K_PS_SPLIT", "3,2,3")  # ps,psO,psS (0=share ps)
OPROJ_DELAY = _os.environ.get("K_OPROJ_DELAY", "1") == "1"
SB_BUFS = 3
AP_BUFS = 6
OB_BUFS = 3
BIG_BUFS = 4
MAX_WAITS = 1  # this container's walrus supports one sync-wait per instruction


class PatchedTileContext(TileContext):
    """Split multi-sem waits into single-wait NOPs (old-walrus limitation)."""

    def _lower_ordered_insts(self, ordered):
        for bb_name, insts in ordered.items():
            new_list = []
            for inst in insts:
                si = inst.sync_info
                if si is not None and len(si.on_wait) > MAX_WAITS:
                    waits = list(si.on_wait)
                    keep = waits[:MAX_WAITS]
                    extra = waits[MAX_WAITS:]
                    scopes = self._inst_to_scopes.get(inst.name, ())
                    for i in range(0, len(extra), MAX_WAITS):
                        group = extra[i:i + MAX_WAITS]
                        nop = mybir.InstNoOp(
                            name=f"waitsplit-{self.nc.next_id()}",
                            engine=inst.engine,
                            sync_info=mybir.SyncInfo(on_wait=list(group), on_update=[]),
                            bass_nofuse=True,
                        )
                        self._inst_to_scopes[nop.name] = scopes
                        new_list.append(nop)
                    inst.sync_info = bass_rust.SyncInfo(
                        on_wait=keep, on_update=list(si.on_update)
                    )
                new_list.append(inst)
            insts[:] = new_list
        return super()._lower_ordered_insts(ordered)

    def _drain_and_barrier(self, tick_clock, wait_clock):
        nc = self.nc
        drain_inst = nc.sync.drain()
        wait_clock.add_sem_waits(
            drain_inst.ins, ScopedClock({None: tick_clock.global_clock})
        )
        si = drain_inst.ins.sync_info
        waits = list(si.on_wait) if si is not None else []
        if len(waits) > MAX_WAITS:
            assert self.sems is not None
            by_name = {h.name: h for h in self.sems.allocated().values()}
            keep = waits[:MAX_WAITS]
            extra = []
            for w in waits[MAX_WAITS:]:
                h = by_name.get(w.ant_name)
                if h is None:
                    keep.append(w)
                else:
                    extra.append((h, w.wait_value, w.wait_mode))
            drain_inst.ins.sync_info = bass_rust.SyncInfo(
                on_wait=keep, on_update=list(si.on_update) if si else []
            )
            for h, val, mode in extra:
                assert mode == "sem-ge-imm", mode
                nc.sync.wait_ge(h, val)

        nc.all_engine_barrier()
        assert self.sems is not None
        popped = nc._tile_sem_poison_stack.pop()
        assert popped is self._sem_poison
        nc.clear_and_free_semaphores(list(self.sems.allocated().values()))
        nc.all_engine_barrier()


def build_kernel(mask_mode: str) -> bass.Bass:
    """mask_mode: 'causal' (skip masked tiles), 'dense' (no mask),
    'generic' (additive mask streamed from DRAM)."""
    nc = bass.Bass()

    hT = nc.dram_tensor("hT", [H, BS], BF16, kind="ExternalInput")
    # m-major packed qk weights: [128, m(4) x kt(16) x 128]
    wqkP = nc.dram_tensor("wqkP", [128, 4 * KT * 128], BF16, kind="ExternalInput")
    wvP = nc.dram_tensor("wvP", [128, KT * DLOC], BF16, kind="ExternalInput")
    owP = nc.dram_tensor("owP", [128, 2 * H], BF16, kind="ExternalInput")
    bqkT = nc.dram_tensor("bqkT", [128, 4], F32, kind="ExternalInput")
    cosT = nc.dram_tensor("cosT", [128, BS], BF16, kind="ExternalInput")
    sinS = nc.dram_tensor("sinS", [128, BS], BF16, kind="ExternalInput")
    permP = nc.dram_tensor("permP", [128, 128], BF16, kind="ExternalInput")
    ones128 = nc.dram_tensor("ones128", [128, 128], BF16, kind="ExternalInput")
    if mask_mode == "causal":
        # additive staircase for the diagonal k-tile pair, [128, 512]:
        #   [0:256]   staircase for k-tile 2qb (0 where k<=q else -1e5)
        #   [256:384] all -1e5 (fully masked q-half of k-tile 2qb+1)
        #   [384:512] staircase for the live q-half of k-tile 2qb+1
        adiagD = nc.dram_tensor("adiag", [128, 512], BF16, kind="ExternalInput")
    if mask_mode == "generic":
        maskT = nc.dram_tensor("maskT", [B, S, S], F32, kind="ExternalInput")
    outP = nc.dram_tensor("outP", [BS, H], BF16, kind="ExternalOutput")

    causal = mask_mode == "causal"
    generic = mask_mode == "generic"

    with PatchedTileContext(nc) as tc:
        with (
            tc.tile_pool(name="const", bufs=1) as cpool,
            tc.tile_pool(name="work", bufs=2) as wpool,
            tc.tile_pool(name="sb", bufs=SB_BUFS) as sb,
            tc.tile_pool(name="bigp", bufs=BIG_BUFS) as bigp,
            tc.tile_pool(name="ap", bufs=AP_BUFS) as apool,
            tc.tile_pool(name="ob", bufs=OB_BUFS) as opool,
            tc.tile_pool(name="mp", bufs=8) as mp,
            # PSUM is bank-granular: 8 banks, split so slow-freed consumers
            # (o_proj evac, scores exp) don't gate unrelated allocations.
            tc.tile_pool(name="ps", bufs=int(PS_SPLIT.split(",")[0]), space="PSUM") as ps,
        ):
            _a, _b, _c = [int(x) for x in PS_SPLIT.split(",")]
            psO_ = tc.tile_pool(name="psO", bufs=_b, space="PSUM").__enter__() if _b else ps
            psS_ = tc.tile_pool(name="psS", bufs=_c, space="PSUM").__enter__() if _c else ps

            # ---- resident constants ----
            # wqk is m-major so each m-column is one contiguous DMA; loads are
            # sliced and interleaved with the first h-chunk groups so the
            # first qkv matmuls start ~2us in.
            wqk_t = cpool.tile([128, 4 * KT * 128], BF16, tag="wqk")
            wv_t = cpool.tile([128, KT * DLOC], BF16, tag="wv")

            def load_wqk_m(m, half=None):
                if half is None:
                    lo, hi = m * 2048, (m + 1) * 2048
                else:
                    lo = m * 2048 + half * 1024
                    hi = lo + 1024
                nc.sync.dma_start(wqk_t[:, lo:hi], wqkP[:, lo:hi])

            def load_wv_half(h):
                lo, hi = h * (KT * DLOC // 2), (h + 1) * (KT * DLOC // 2)
                nc.sync.dma_start(wv_t[:, lo:hi], wvP[:, lo:hi])

            load_wqk_m(0, half=0)
            bqk_t = cpool.tile([128, 4], F32, tag="bqk")
            nc.sync.dma_start(bqk_t[:], bqkT[:, :])
            perm_t = cpool.tile([128, 128], BF16, tag="perm")
            nc.sync.dma_start(perm_t[:], permP[:, :])

            # late-loaded constants (first needed by attention q-block 0)
            ow_t = cpool.tile([128, 2 * H], BF16, tag="ow")
            ones_t = cpool.tile([128, 128], BF16, tag="ones")
            if causal:
                adiag_t = cpool.tile([128, 512], BF16, tag="adiag")

            def load_small_consts():
                nc.sync.dma_start(ones_t[:], ones128[:, :])
                if causal:
                    nc.sync.dma_start(adiag_t[:], adiagD[:, :])

            ow_loaded = [False]

            def load_ow():
                ow_loaded[0] = True
                nc.sync.dma_start(ow_t[:, 0:H], owP[:, 0:H])
                nc.sync.dma_start(ow_t[:, H:2 * H], owP[:, H:2 * H])

            for b in range(B):
                s_base = b * S
                qk_t = wpool.tile([128, 4 * S], BF16, tag="qkT")   # 4 m x [128,S]
                v_t = wpool.tile([128, KT * DLOC], BF16, tag="v")  # KT s-tiles
                cos_t = wpool.tile([128, S], BF16, tag="cos")
                sin_t = wpool.tile([128, S], BF16, tag="sin")

                def load_trig(half):
                    h0 = half * (S // 2)
                    nc.sync.dma_start(
                        cos_t[:, h0:h0 + S // 2],
                        cosT[:, s_base + h0:s_base + h0 + S // 2])
                    nc.sync.dma_start(
                        sin_t[:, h0:h0 + S // 2],
                        sinS[:, s_base + h0:s_base + h0 + S // 2])

                def load_chunk(n, interleave=()):
                    """Load h chunk n; optionally interleave other DMAs
                    between the kt-group DMAs (startup pipelining)."""
                    s0 = s_base + n * CH
                    h_t = bigp.tile([128, KT * CH], BF16, tag="big")
                    step = 4 if interleave else 8
                    il = list(interleave)
                    for ktg in range(0, KT, step):
                        nc.sync.dma_start(
                            h_t[:, ktg * CH:(ktg + step) * CH]
                            .rearrange("p (kt s) -> p kt s", kt=step),
                            hT[ktg * 128:(ktg + step) * 128, s0:s0 + CH]
                            .rearrange("(kt p) s -> p kt s", p=128),
                        )
                        if il:
                            il.pop(0)()
                    for fn in il:
                        fn()
                    return h_t

                def do_qkv_chunk(n, h_t):
                    if n + 1 < NCH:
                        # prefetch the next chunk so its DMA overlaps this
                        # chunk's compute and the sem fires well before use
                        h_tiles[n + 1] = load_chunk(n + 1)
                    if n == 0 and b == 0:
                        load_small_consts()
                        load_trig(0)
                    if n == 1 and not ow_loaded[0]:
                        load_ow()
                    if n == NCH // 2 - 1:
                        load_trig(1)
                    for m in range(4):  # q_h0,q_h1,k_h0,k_h1
                        p_qk = ps.tile([128, 512], F32, tag="ps")
                        for kt in range(KT):
                            nc.tensor.matmul(
                                p_qk[:, 0:256],
                                wqk_t[:, m * 2048 + kt * 128: m * 2048 + (kt + 1) * 128],
                                h_t[:, kt * CH:(kt + 1) * CH],
                                start=(kt == 0), stop=(kt == KT - 1),
                            )
                        nc.scalar.activation(
                            qk_t[:, m * S + n * CH: m * S + (n + 1) * CH],
                            p_qk[:, 0:256], AF.Identity, bias=bqk_t[:, m:m + 1],
                        )
                    for st in range(2):  # v in [s, d]
                        p_v = ps.tile([128, 512], F32, tag="ps")
                        for kt in range(KT):
                            nc.tensor.matmul(
                                p_v[:, 0:256],
                                h_t[:, kt * CH + st * 128: kt * CH + (st + 1) * 128],
                                wv_t[:, kt * DLOC:(kt + 1) * DLOC],
                                start=(kt == 0), stop=(kt == KT - 1),
                            )
                        stile = n * 2 + st
                        nc.scalar.activation(
                            v_t[:, stile * DLOC:(stile + 1) * DLOC], p_v[:, 0:256],
                            AF.Copy,
                        )

                def do_rope_chunk(n):
                    c0 = n * CH
                    for m in range(4):
                        qk_sl = qk_t[:, m * S + c0: m * S + c0 + CH]
                        p_rot = ps.tile([128, 512], F32, tag="ps")
                        nc.tensor.matmul(
                            p_rot[:, 0:256], perm_t[:], qk_sl, start=True, stop=True
                        )
                        rot_sb = sb.tile([128, CH], BF16, tag="rot")
                        # fused evac: rot_sb = psum_rot * sinS  (DVE reads PSUM)
                        nc.vector.tensor_tensor(
                            rot_sb[:], p_rot[:, 0:256], sin_t[:, c0:c0 + CH], OP.mult
                        )
                        t2 = sb.tile([128, CH], BF16, tag="t2")
                        nc.vector.tensor_tensor(
                            t2[:], qk_sl, cos_t[:, c0:c0 + CH], OP.mult
                        )
                        nc.vector.tensor_tensor(qk_sl, t2[:], rot_sb[:], OP.add)

                def attention_stage_a(qb):
                    """scores -> exp -> sum -> PV -> 1/sum -> at tiles.
                    Both heads' scores+exp are emitted first so each head's
                    exp tail drains under the other head's PE work."""
                    n_kt = 2 * (qb + 1) if causal else KT
                    if generic:
                        mask_tiles = []
                        for pair in range(n_kt // 2):
                            mt = mp.tile([128, 512], F32, tag="mask")
                            nc.sync.dma_start(
                                mt[:].rearrange("p (t q) -> p t q", t=2),
                                maskT[b, pair * 256:(pair + 1) * 256,
                                      qb * CH:(qb + 1) * CH]
                                .rearrange("(t p) q -> p t q", p=128),
                            )
                            mask_tiles.append(mt)
                    ex_tiles = []
                    for hh in range(HPC):
                        qof = hh * S
                        kof = (2 + hh) * S
                        q_full = qk_t[:, qof + qb * CH: qof + (qb + 1) * CH]
                        ex_t = bigp.tile([128, KT * CH], BF16, tag="big")
                        for pair in range((n_kt + 1) // 2):
                            kt0 = 2 * pair
                            diag = causal and (pair == qb)
                            p_sc = psS_.tile([128, 512], F32, tag="ps2" if psS_ is not ps else "ps")
                            if diag:
                                # pre-write additive staircase; score matmuls
                                # accumulate on top (start=False; Pool engine,
                                # off the exp->PV critical path)
                                nc.gpsimd.tensor_copy(p_sc[:], adiag_t[:])
                            for gi in range(2):
                                kt = kt0 + gi
                                if kt >= n_kt:
                                    break
                                k_sl = qk_t[:, kof + kt * 128: kof + (kt + 1) * 128]
                                if diag and gi == 1:
                                    # only the live q-half of the 2nd diag tile
                                    nc.tensor.matmul(
                                        p_sc[:, 384:512], k_sl,
                                        qk_t[:, qof + qb * CH + 128:
                                             qof + (qb + 1) * CH],
                                        start=False, stop=True,
                                        skip_group_check=True,
                                    )
                                else:
                                    nc.tensor.matmul(
                                        p_sc[:, gi * CH:(gi + 1) * CH],
                                        k_sl, q_full,
                                        start=not diag, stop=True,
                                        skip_group_check=True,
                                    )
                                if generic:
                                    mt = mask_tiles[pair]
                                    nc.vector.tensor_tensor(
                                        p_sc[:, gi * CH:(gi + 1) * CH],
                                        p_sc[:, gi * CH:(gi + 1) * CH],
                                        mt[:, gi * CH:(gi + 1) * CH], OP.add,
                                    )
                            w = min(2, n_kt - kt0)
                            nc.scalar.activation(
                                ex_t[:, kt0 * CH:(kt0 + w) * CH],
                                p_sc[:, 0:w * CH], AF.Exp, scale=SCALE,
                            )
                        ex_tiles.append(ex_t)
                    at_tiles = []
                    for hh in range(HPC):
                        ex_t = ex_tiles[hh]
                        # softmax denominator: ones-matmul partition reduction
                        p_sum = ps.tile([128, 512], F32, tag="ps")
                        for kt in range(n_kt):
                            half = causal and (kt == 2 * qb + 1)
                            nc.tensor.matmul(
                                p_sum[:, 128:256] if half else p_sum[:, 0:256],
                                ones_t[:],
                                ex_t[:, kt * CH + (128 if half else 0):
                                     (kt + 1) * CH],
                                start=(kt == 0), stop=(kt == n_kt - 1),
                                skip_group_check=True,
                            )
                        # PV
                        p_at = ps.tile([128, 512], F32, tag="ps")
                        for kt in range(n_kt):
                            half = causal and (kt == 2 * qb + 1)
                            nc.tensor.matmul(
                                p_at[:, 128:256] if half else p_at[:, 0:256],
                                v_t[:, kt * DLOC + hh * 128: kt * DLOC + (hh + 1) * 128],
                                ex_t[:, kt * CH + (128 if half else 0):
                                     (kt + 1) * CH],
                                start=(kt == 0), stop=(kt == n_kt - 1),
                                skip_group_check=True,
                            )
                        # 1/sum (all 128 partition rows of p_sum are identical)
                        rec_t = sb.tile([128, CH], F32, tag="rec")
                        nc.vector.reciprocal(rec_t[:], p_sum[:, 0:256])
                        at_t = apool.tile([128, CH], BF16, tag="attn")
                        nc.vector.tensor_tensor(at_t[:], p_at[:, 0:256], rec_t[:], OP.mult)
                        at_tiles.append(at_t)
                    return at_tiles

                def do_oproj(qb, at_tiles):
                    # o_proj for this q-block; stage two 512-e chunks per
                    # [128,1024] tile so output DMAs stay large
                    last_block = (b == B - 1) and (qb == NCH - 1)
                    for ss in range(CH // 128):
                        for eg in range(H // 1024):
                            # the big pool's slots are idle at the kernel tail;
                            # borrow them so the final stores pipeline deeper
                            if last_block:
                                o_sb = bigp.tile([128, 1024], BF16, tag="big")
                            else:
                                o_sb = opool.tile([128, 1024], BF16, tag="osb")
                            for sub in range(2):
                                ec = eg * 2 + sub
                                p_o = psO_.tile([128, 512], F32, tag="po" if psO_ is not ps else "ps")
                                for hh in range(HPC):
                                    nc.tensor.matmul(
                                        p_o[:],
                                        at_tiles[hh][:, ss * 128:(ss + 1) * 128],
                                        ow_t[:, hh * H + ec * 512: hh * H + (ec + 1) * 512],
                                        start=(hh == 0), stop=(hh == HPC - 1),
                                    )
                                dst = o_sb[:, sub * 512:(sub + 1) * 512]
                                if (ss * 2 + eg) % 2 == 0:
                                    nc.vector.tensor_copy(dst, p_o[:])
                                else:
                                    nc.gpsimd.tensor_copy(dst, p_o[:])
                            nc.sync.dma_start(
                                outP[s_base + qb * CH + ss * 128:
                                     s_base + qb * CH + (ss + 1) * 128,
                                     eg * 1024:(eg + 1) * 1024],
                                o_sb[:],
                            )

                h_tiles = {}
                if b == 0:
                    h_tiles[0] = load_chunk(0, interleave=[
                        lambda: load_wqk_m(0, half=1),
                        lambda: load_wqk_m(1),
                        lambda: (load_wqk_m(2), load_wqk_m(3)),
                        lambda: (load_wv_half(0), load_wv_half(1)),
                    ])
                else:
                    h_tiles[0] = load_chunk(0)
                if causal:
                    # software pipeline: attention stage A (scores..at) for
                    # q-block n-1 is emitted with chunk n; its o_proj is
                    # delayed one more slot so the at-mult chain drains under
                    # chunk n+1's qkv matmuls.
                    pend = {}
                    for n in range(NCH):
                        do_qkv_chunk(n, h_tiles.pop(n))
                        do_rope_chunk(n)
                        if OPROJ_DELAY:
                            if n - 1 - LAG >= 0:
                                do_oproj(n - 1 - LAG, pend.pop(n - 1 - LAG))
                            if n >= LAG:
                                pend[n - LAG] = attention_stage_a(n - LAG)
                        elif n >= LAG:
                            do_oproj(n - LAG, attention_stage_a(n - LAG))
                    if OPROJ_DELAY:
                        for qb in range(NCH - LAG - 1, NCH):
                            if qb in pend:
                                do_oproj(qb, pend.pop(qb))
                            if qb + LAG < NCH:
                                pend[qb + LAG] = attention_stage_a(qb + LAG)
                    else:
                        for t in range(LAG):
                            do_oproj(NCH - LAG + t,
                                     attention_stage_a(NCH - LAG + t))
                else:
                    for n in range(NCH):
                        do_qkv_chunk(n, h_tiles.pop(n))
                    for n in range(NCH):
                        do_rope_chunk(n)
                    for qb in range(NCH):
                        do_oproj(qb, attention_stage_a(qb))
    return nc


def _adiag_pattern():
    p = np.arange(128)[:, None]
    j = np.arange(512)[None, :]
    out = np.zeros((128, 512), dtype=np.float32)
    out[:, 0:256] = np.where(p <= j[:, 0:256], 0.0, MASK_NEG)
    out[:, 256:384] = MASK_NEG
    out[:, 384:512] = np.where(p <= (j[:, 384:512] - 384), 0.0, MASK_NEG)
    return out


def _host_prep(hidden_states, position_ids, attention_mask, qkv_w, qkv_b, o_w):
    import ml_dtypes
    bf16 = ml_dtypes.bfloat16

    hidden_states = np.asarray(hidden_states, dtype=np.float32)
    position_ids = np.asarray(position_ids)
    attention_mask = np.asarray(attention_mask, dtype=np.float32)
    qkv_w = np.asarray(qkv_w, dtype=np.float32)
    qkv_b = np.asarray(qkv_b, dtype=np.float32)
    o_w = np.asarray(o_w, dtype=np.float32)

    # mask mode detection
    causal = np.triu(np.full((S, S), -1e9, dtype=np.float32), k=1)
    m = attention_mask.reshape(B, S, S)
    if all(np.array_equal(m[b], causal) for b in range(B)):
        mask_mode = "causal"
    elif not attention_mask.any():
        mask_mode = "dense"
    else:
        mask_mode = "generic"

    # rope tables
    half = HD // 2
    inv = (1.0 / ROPE_BASE ** (np.arange(half, dtype=np.float64) / half))
    freqs = position_ids.astype(np.float64).reshape(BS, 1) * inv[None, :]  # [BS,64]
    c = np.cos(freqs).T  # [64, BS]
    s_ = np.sin(freqs).T
    cosT = np.concatenate([c, c], 0).astype(bf16)
    sinS = np.concatenate([-s_, s_], 0).astype(bf16)

    hT = np.ascontiguousarray(hidden_states.reshape(BS, H).T).astype(bf16)

    perm = np.zeros((128, 128), dtype=np.float32)
    for dp in range(128):
        perm[(dp + 64) % 128, dp] = 1.0  # out[dp] = in[(dp+64)%128]

    shared = {
        "hT": hT, "cosT": cosT, "sinS": sinS,
        "permP": perm.astype(bf16),
        "ones128": np.ones((128, 128), dtype=bf16),
    }
    if mask_mode == "causal":
        shared["adiag"] = _adiag_pattern().astype(bf16)
    if mask_mode == "generic":
        shared["maskT"] = np.ascontiguousarray(
            np.transpose(m, (0, 2, 1)) / SCALE
        ).astype(np.float32)

    in_maps = []
    for c_id in range(8):
        r = c_id * DLOC
        wqk = np.vstack([qkv_w[r:r + DLOC], qkv_w[H + r:H + r + DLOC]])      # [512, H]
        wv = qkv_w[2 * H + r: 2 * H + r + DLOC]                               # [256, H]
        bqk = np.concatenate([qkv_b[r:r + DLOC], qkv_b[H + r:H + r + DLOC]])  # [512]
        im = dict(shared)
        # m-major packing: wqkP[p, m*KT*128 + kt*128 + c] = wqk[m*128+c, kt*128+p]
        wqkT = np.ascontiguousarray(wqk.T)                 # [H, 512]
        im["wqkP"] = np.ascontiguousarray(
            wqkT.reshape(KT, 128, 4, 128).transpose(1, 2, 0, 3).reshape(128, -1)
        ).astype(bf16)
        wvT = np.ascontiguousarray(wv.T)                   # [H, 256]
        im["wvP"] = np.ascontiguousarray(
            wvT.reshape(KT, 128, DLOC).transpose(1, 0, 2).reshape(128, -1)
        ).astype(bf16)
        owT = np.ascontiguousarray(o_w[:, r:r + DLOC].T)   # [256, H]
        im["owP"] = np.ascontiguousarray(
            owT.reshape(2, 128, H).transpose(1, 0, 2).reshape(128, -1)
        ).astype(bf16)
        im["bqkT"] = np.ascontiguousarray(bqk.reshape(4, 128).T)
        in_maps.append(im)
    post_bias = qkv_b[2 * H:3 * H] @ o_w.T  # [H], exact since sum(probs)=1
    return mask_mode, in_maps, post_bias


def kernel(**inputs) -> np.ndarray:
    import os
    import sys
    # The devices are reached through the axon PJRT proxy; make sure a
    # JAX_PLATFORMS=cpu pin (used for CPU-side reference runs) doesn't hide
    # them if jax hasn't been imported yet.
    if os.environ.get("JAX_PLATFORMS") == "cpu" and "jax" not in sys.modules:
        del os.environ["JAX_PLATFORMS"]
    from concourse.bass_utils import run_bass_kernel_spmd

    mask_mode, in_maps, post_bias = _host_prep(**inputs)
    nc = build_kernel(mask_mode)
    res = run_bass_kernel_spmd(nc, in_maps, core_ids=list(range(8)), trace=False)
    out = np.zeros((BS, H), dtype=np.float64)
    for r in res.results:
        out += r["outP"].astype(np.float64)
    out += post_bias.astype(np.float64)[None, :]
    return out.astype(np.float32).reshape(B, S, H)


# revision 16
# speedup vs baseline: 1.0300x; 1.0109x over previous
"""TRN2 Bass kernel for nn_BaseAttention_46548855554192.

B=2, S=2048, H=2048, NH=16, HD=128 multi-head attention with RoPE and an
additive attention mask, computed tensor-parallel over heads on 8 NeuronCores
(2 heads per core).  Each core computes the qkv projection for its heads,
RoPE, causal softmax attention, and a partial o_proj (its head-columns of
o_w); the host sums the 8 partial outputs.

Layout strategy (per core):
  - hidden is fed transposed hT [H, B*S]; q,k computed as [feat, s] so the
    head dim (128) lands on SBUF partitions; v computed in [s, d] directly.
  - scores are computed transposed (scoresT [k, q]) so the softmax sum is a
    partition-dim reduction done on the PE (ones-matmul) and exp is a
    PSUM->SBUF ACT op; 1/sum comes from DVE reciprocal broadcast across
    partitions by the ones-matmul itself.
  - no max-subtraction in softmax (scores are O(10) for randn inputs; exp in
    f32 is safe to ~88; mathematically identical to the reference).
  - RoPE rotate_half is a permutation matmul (engines cannot move data
    across partitions); cos/sin tables are host-side (ACT Sin has no range
    reduction) with the rotation sign folded into the sin table.
  - causal masking: fully-masked k-tiles are skipped; the two diagonal
    k-tiles get an additive -1e5 staircase PRE-WRITTEN into PSUM (DVE copy,
    off the critical path) and the score matmuls accumulate on top, so no
    post-exp mask op sits between exp and the sum/PV matmuls.  The fully
    masked q-half of the second diagonal tile is skipped in scores/sum/PV.
  - causal mode interleaves qkv chunk n -> rope n -> attention q-block n so
    both batches stream without a serialization point.
  - everything streamed is bf16 (matmul rate at >=256 cols equals fp32r, but
    DMA bytes halve, DVE elementwise gets 2x, SBUF pressure halves); PSUM
    accumulation stays f32 except single-shot score/rope tiles (bf16).
"""

import numpy as np

import bass_rust
import concourse.bass as bass
import concourse.mybir as mybir
from concourse.tile import TileContext
from concourse.vector_clock import ScopedClock

F32 = mybir.dt.float32
BF16 = mybir.dt.bfloat16
AF = mybir.ActivationFunctionType
OP = mybir.AluOpType

B, S, H, NH, HD = 2, 2048, 2048, 16, 128
BS = B * S                  # 4096
HPC = NH // 8               # heads per core = 2
DLOC = HPC * HD             # local attn dims = 256
CH = 256                    # chunk / q-block width
NCH = S // CH               # 8 chunks per batch
KT = S // 128               # 16 k-tiles of 128 per batch
SCALE = 1.0 / float(np.sqrt(HD))
ROPE_BASE = 10000.0
MASK_NEG = -1.0e5           # additive mask; exp(SCALE*-1e5) == 0 in f32

LAG = 1
import os as _os
PS_SPLIT = _os.environ.get("K_PS_SPLIT", "3,2,3")  # ps,psO,psS (0=share ps)
OPROJ_DELAY = _os.environ.get("K_OPROJ_DELAY", "1") == "1"
SB_BUFS = 3
AP_BUFS = 6
OB_BUFS = 3
BIG_BUFS = 4
MAX_WAITS = 1  # this container's walrus supports one sync-wait per instruction


class PatchedTileContext(TileContext):
    """Split multi-sem waits into single-wait NOPs (old-walrus limitation)."""

    def _lower_ordered_insts(self, ordered):
        for bb_name, insts in ordered.items():
            new_list = []
            for inst in insts:
                si = inst.sync_info
                if si is not None and len(si.on_wait) > MAX_WAITS:
                    waits = list(si.on_wait)
                    keep = waits[:MAX_WAITS]
                    extra = waits[MAX_WAITS:]
                    scopes = self._inst_to_scopes.get(inst.name, ())
                    for i in range(0, len(extra), MAX_WAITS):
                        group = extra[i:i + MAX_WAITS]
                        nop = mybir.InstNoOp(
                            name=f"waitsplit-{self.nc.next_id()}",
                            engine=inst.engine,
                            sync_info=mybir.SyncInfo(on_wait=list(group), on_update=[]),
                            bass_nofuse=True,
                        )
                        self._inst_to_scopes[nop.name] = scopes
                        new_list.append(nop)
                    inst.sync_info = bass_rust.SyncInfo(
                        on_wait=keep, on_update=list(si.on_update)
                    )
                new_list.append(inst)
            insts[:] = new_list
        return super()._lower_ordered_insts(ordered)

    def _drain_and_barrier(self, tick_clock, wait_clock):
        nc = self.nc
        drain_inst = nc.sync.drain()
        wait_clock.add_sem_waits(
            drain_inst.ins, ScopedClock({None: tick_clock.global_clock})
        )
        si = drain_inst.ins.sync_info
        waits = list(si.on_wait) if si is not None else []
        if len(waits) > MAX_WAITS:
            assert self.sems is not None
            by_name = {h.name: h for h in self.sems.allocated().values()}
            keep = waits[:MAX_WAITS]
            extra = []
            for w in waits[MAX_WAITS:]:
                h = by_name.get(w.ant_name)
                if h is None:
                    keep.append(w)
                else:
                    extra.append((h, w.wait_value, w.wait_mode))
            drain_inst.ins.sync_info = bass_rust.SyncInfo(
                on_wait=keep, on_update=list(si.on_update) if si else []
            )
            for h, val, mode in extra:
                assert mode == "sem-ge-imm", mode
                nc.sync.wait_ge(h, val)

        nc.all_engine_barrier()
        assert self.sems is not None
        popped = nc._tile_sem_poison_stack.pop()
        assert popped is self._sem_poison
        nc.clear_and_free_semaphores(list(self.sems.allocated().values()))
        nc.all_engine_barrier()


def build_kernel(mask_mode: str) -> bass.Bass:
    """mask_mode: 'causal' (skip masked tiles), 'dense' (no mask),
    'generic' (additive mask streamed from DRAM)."""
    nc = bass.Bass()

    hT = nc.dram_tensor("hT", [H, BS], BF16, kind="ExternalInput")
    # m-major packed qk weights: [128, m(4) x kt(16) x 128]
    wqkP = nc.dram_tensor("wqkP", [128, 4 * KT * 128], BF16, kind="ExternalInput")
    wvP = nc.dram_tensor("wvP", [128, KT * DLOC], BF16, kind="ExternalInput")
    owP = nc.dram_tensor("owP", [128, 2 * H], BF16, kind="ExternalInput")
    bqkT = nc.dram_tensor("bqkT", [128, 4], F32, kind="ExternalInput")
    cosT = nc.dram_tensor("cosT", [128, BS], BF16, kind="ExternalInput")
    sinS = nc.dram_tensor("sinS", [128, BS], BF16, kind="ExternalInput")
    permP = nc.dram_tensor("permP", [128, 128], BF16, kind="ExternalInput")
    ones128 = nc.dram_tensor("ones128", [128, 128], BF16, kind="ExternalInput")
    if mask_mode == "causal":
        # additive staircase for the diagonal k-tile pair, [128, 512]:
        #   [0:256]   staircase for k-tile 2qb (0 where k<=q else -1e5)
        #   [256:384] all -1e5 (fully masked q-half of k-tile 2qb+1)
        #   [384:512] staircase for the live q-half of k-tile 2qb+1
        adiagD = nc.dram_tensor("adiag", [128, 512], BF16, kind="ExternalInput")
    if mask_mode == "generic":
        maskT = nc.dram_tensor("maskT", [B, S, S], F32, kind="ExternalInput")
    outP = nc.dram_tensor("outP", [BS, H], BF16, kind="ExternalOutput")

    causal = mask_mode == "causal"
    generic = mask_mode == "generic"

    with PatchedTileContext(nc) as tc:
        with (
            tc.tile_pool(name="const", bufs=1) as cpool,
            tc.tile_pool(name="work", bufs=2) as wpool,
            tc.tile_pool(name="sb", bufs=SB_BUFS) as sb,
            tc.tile_pool(name="bigp", bufs=BIG_BUFS) as bigp,
            tc.tile_pool(name="ap", bufs=AP_BUFS) as apool,
            tc.tile_pool(name="ob", bufs=OB_BUFS) as opool,
            tc.tile_pool(name="mp", bufs=8) as mp,
            # PSUM is bank-granular: 8 banks, split so slow-freed consumers
            # (o_proj evac, scores exp) don't gate unrelated allocations.
            tc.tile_pool(name="ps", bufs=int(PS_SPLIT.split(",")[0]), space="PSUM") as ps,
        ):
            _a, _b, _c = [int(x) for x in PS_SPLIT.split(",")]
            psO_ = tc.tile_pool(name="psO", bufs=_b, space="PSUM").__enter__() if _b else ps
            psS_ = tc.tile_pool(name="psS", bufs=_c, space="PSUM").__enter__() if _c else ps

            # ---- resident constants ----
            # wqk is m-major so each m-column is one contiguous DMA; loads are
            # sliced and interleaved with the first h-chunk groups so the
            # first qkv matmuls start ~2us in.
            wqk_t = cpool.tile([128, 4 * KT * 128], BF16, tag="wqk")
            wv_t = cpool.tile([128, KT * DLOC], BF16, tag="wv")

            def load_wqk_m(m, half=None):
                if half is None:
                    lo, hi = m * 2048, (m + 1) * 2048
                else:
                    lo = m * 2048 + half * 1024
                    hi = lo + 1024
                nc.sync.dma_start(wqk_t[:, lo:hi], wqkP[:, lo:hi])

            def load_wv_half(h):
                lo, hi = h * (KT * DLOC // 2), (h + 1) * (KT * DLOC // 2)
                nc.sync.dma_start(wv_t[:, lo:hi], wvP[:, lo:hi])

            load_wqk_m(0, half=0)
            bqk_t = cpool.tile([128, 4], F32, tag="bqk")
            nc.sync.dma_start(bqk_t[:], bqkT[:, :])
            perm_t = cpool.tile([128, 128], BF16, tag="perm")
            nc.sync.dma_start(perm_t[:], permP[:, :])

            # late-loaded constants (first needed by attention q-block 0)
            ow_t = cpool.tile([128, 2 * H], BF16, tag="ow")
            ones_t = cpool.tile([128, 128], BF16, tag="ones")
            if causal:
                adiag_t = cpool.tile([128, 512], BF16, tag="adiag")

            def load_small_consts():
                nc.sync.dma_start(ones_t[:], ones128[:, :])
                if causal:
                    nc.sync.dma_start(adiag_t[:], adiagD[:, :])

            ow_loaded = [False]

            def load_ow():
                ow_loaded[0] = True
                nc.sync.dma_start(ow_t[:, 0:H], owP[:, 0:H])
                nc.sync.dma_start(ow_t[:, H:2 * H], owP[:, H:2 * H])

            for b in range(B):
                s_base = b * S
                qk_t = wpool.tile([128, 4 * S], BF16, tag="qkT")   # 4 m x [128,S]
                v_t = wpool.tile([128, KT * DLOC], BF16, tag="v")  # KT s-tiles
                cos_t = wpool.tile([128, S], BF16, tag="cos")
                sin_t = wpool.tile([128, S], BF16, tag="sin")

                def load_trig(half):
                    h0 = half * (S // 2)
                    nc.sync.dma_start(
                        cos_t[:, h0:h0 + S // 2],
                        cosT[:, s_base + h0:s_base + h0 + S // 2])
                    nc.sync.dma_start(
                        sin_t[:, h0:h0 + S // 2],
                        sinS[:, s_base + h0:s_base + h0 + S // 2])

                def load_chunk(n, interleave=()):
                    """Load h chunk n; optionally interleave other DMAs
                    between the kt-group DMAs (startup pipelining)."""
                    s0 = s_base + n * CH
                    h_t = bigp.tile([128, KT * CH], BF16, tag="big")
                    step = 4 if interleave else 8
                    il = list(interleave)
                    for ktg in range(0, KT, step):
                        nc.sync.dma_start(
                            h_t[:, ktg * CH:(ktg + step) * CH]
                            .rearrange("p (kt s) -> p kt s", kt=step),
                            hT[ktg * 128:(ktg + step) * 128, s0:s0 + CH]
                            .rearrange("(kt p) s -> p kt s", p=128),
                        )
                        if il:
                            il.pop(0)()
                    for fn in il:
                        fn()
                    return h_t

                def do_qkv_chunk(n, h_t):
                    if n + 1 < NCH:
                        # prefetch the next chunk so its DMA overlaps this
                        # chunk's compute and the sem fires well before use
                        h_tiles[n + 1] = load_chunk(n + 1)
                    if n == 0 and b == 0:
                        load_small_consts()
                        load_trig(0)
                    if n == 1 and not ow_loaded[0]:
                        load_ow()
                    if n == NCH // 2 - 1:
                        load_trig(1)
                    for m in range(4):  # q_h0,q_h1,k_h0,k_h1
                        p_qk = ps.tile([128, 512], F32, tag="ps")
                        for kt in range(KT):
                            nc.tensor.matmul(
                                p_qk[:, 0:256],
                                wqk_t[:, m * 2048 + kt * 128: m * 2048 + (kt + 1) * 128],
                                h_t[:, kt * CH:(kt + 1) * CH],
                                start=(kt == 0), stop=(kt == KT - 1),
                            )
                        nc.scalar.activation(
                            qk_t[:, m * S + n * CH: m * S + (n + 1) * CH],
                            p_qk[:, 0:256], AF.Identity, bias=bqk_t[:, m:m + 1],
                        )
                    for st in range(2):  # v in [s, d]
                        p_v = ps.tile([128, 512], F32, tag="ps")
                        for kt in range(KT):
                            nc.tensor.matmul(
                                p_v[:, 0:256],
                                h_t[:, kt * CH + st * 128: kt * CH + (st + 1) * 128],
                                wv_t[:, kt * DLOC:(kt + 1) * DLOC],
                                start=(kt == 0), stop=(kt == KT - 1),
                            )
                        stile = n * 2 + st
                        nc.scalar.activation(
                            v_t[:, stile * DLOC:(stile + 1) * DLOC], p_v[:, 0:256],
                            AF.Copy,
                        )

                def do_rope_chunk(n):
                    c0 = n * CH
                    for m in range(4):
                        qk_sl = qk_t[:, m * S + c0: m * S + c0 + CH]
                        p_rot = ps.tile([128, 512], F32, tag="ps")
                        nc.tensor.matmul(
                            p_rot[:, 0:256], perm_t[:], qk_sl, start=True, stop=True
                        )
                        rot_sb = sb.tile([128, CH], BF16, tag="rot")
                        # fused evac: rot_sb = psum_rot * sinS  (DVE reads PSUM)
                        nc.vector.tensor_tensor(
                            rot_sb[:], p_rot[:, 0:256], sin_t[:, c0:c0 + CH], OP.mult
                        )
                        t2 = sb.tile([128, CH], BF16, tag="t2")
                        nc.vector.tensor_tensor(
                            t2[:], qk_sl, cos_t[:, c0:c0 + CH], OP.mult
                        )
                        nc.vector.tensor_tensor(qk_sl, t2[:], rot_sb[:], OP.add)

                def attention_stage_a(qb):
                    """scores -> exp -> sum -> PV -> 1/sum -> at tiles.
                    Both heads' scores+exp are emitted first so each head's
                    exp tail drains under the other head's PE work."""
                    n_kt = 2 * (qb + 1) if causal else KT
                    if generic:
                        mask_tiles = []
                        for pair in range(n_kt // 2):
                            mt = mp.tile([128, 512], F32, tag="mask")
                            nc.sync.dma_start(
                                mt[:].rearrange("p (t q) -> p t q", t=2),
                                maskT[b, pair * 256:(pair + 1) * 256,
                                      qb * CH:(qb + 1) * CH]
                                .rearrange("(t p) q -> p t q", p=128),
                            )
                            mask_tiles.append(mt)
                    at_tiles = []
                    for hh in range(HPC):
                        qof = hh * S
                        kof = (2 + hh) * S
                        q_full = qk_t[:, qof + qb * CH: qof + (qb + 1) * CH]
                        ex_t = bigp.tile([128, KT * CH], BF16, tag="big")
                        for pair in range((n_kt + 1) // 2):
                            kt0 = 2 * pair
                            diag = causal and (pair == qb)
                            p_sc = psS_.tile([128, 512], F32, tag="ps2" if psS_ is not ps else "ps")
                            if diag:
                                # pre-write additive staircase; score matmuls
                                # accumulate on top (start=False; Pool engine,
                                # off the exp->PV critical path)
                                nc.gpsimd.tensor_copy(p_sc[:], adiag_t[:])
                            for gi in range(2):
                                kt = kt0 + gi
                                if kt >= n_kt:
                                    break
                                k_sl = qk_t[:, kof + kt * 128: kof + (kt + 1) * 128]
                                if diag and gi == 1:
                                    # only the live q-half of the 2nd diag tile
                                    nc.tensor.matmul(
                                        p_sc[:, 384:512], k_sl,
                                        qk_t[:, qof + qb * CH + 128:
                                             qof + (qb + 1) * CH],
                                        start=False, stop=True,
                                        skip_group_check=True,
                                    )
                                else:
                                    nc.tensor.matmul(
                                        p_sc[:, gi * CH:(gi + 1) * CH],
                                        k_sl, q_full,
                                        start=not diag, stop=True,
                                        skip_group_check=True,
                                    )
                                if generic:
                                    mt = mask_tiles[pair]
                                    nc.vector.tensor_tensor(
                                        p_sc[:, gi * CH:(gi + 1) * CH],
                                        p_sc[:, gi * CH:(gi + 1) * CH],
                                        mt[:, gi * CH:(gi + 1) * CH], OP.add,
                                    )
                            w = min(2, n_kt - kt0)
                            nc.scalar.activation(
                                ex_t[:, kt0 * CH:(kt0 + w) * CH],
                                p_sc[:, 0:w * CH], AF.Exp, scale=SCALE,
                            )
                        # softmax denominator: ones-matmul partition reduction
                        p_sum = ps.tile([128, 512], F32, tag="ps")
                        for kt in range(n_kt):
                            half = causal and (kt == 2 * qb + 1)
                            nc.tensor.matmul(
                                p_sum[:, 128:256] if half else p_sum[:, 0:256],
                                ones_t[:],
                                ex_t[:, kt * CH + (128 if half else 0):
                                     (kt + 1) * CH],
                                start=(kt == 0), stop=(kt == n_kt - 1),
                                skip_group_check=True,
                            )
                        # PV
                        p_at = ps.tile([128, 512], F32, tag="ps")
                        for kt in range(n_kt):
                            half = causal and (kt == 2 * qb + 1)
                            nc.tensor.matmul(
                                p_at[:, 128:256] if half else p_at[:, 0:256],
                                v_t[:, kt * DLOC + hh * 128: kt * DLOC + (hh + 1) * 128],
                                ex_t[:, kt * CH + (128 if half else 0):
                                     (kt + 1) * CH],
                                start=(kt == 0), stop=(kt == n_kt - 1),
                                skip_group_check=True,
                            )
                        # 1/sum (all 128 partition rows of p_sum are identical)
                        rec_t = sb.tile([128, CH], F32, tag="rec")
                        nc.vector.reciprocal(rec_t[:], p_sum[:, 0:256])
                        at_t = apool.tile([128, CH], BF16, tag="attn")
                        nc.vector.tensor_tensor(at_t[:], p_at[:, 0:256], rec_t[:], OP.mult)
                        at_tiles.append(at_t)
                    return at_tiles

                def do_oproj(qb, at_tiles):
                    # o_proj for this q-block; stage two 512-e chunks per
                    # [128,1024] tile so output DMAs stay large
                    last_block = (b == B - 1) and (qb == NCH - 1)
                    for ss in range(CH // 128):
                        for eg in range(H // 1024):
                            # the big pool's slots are idle at the kernel tail;
                            # borrow them so the final stores pipeline deeper
                            if last_block:
                                o_sb = bigp.tile([128, 1024], BF16, tag="big")
                            else:
                                o_sb = opool.tile([128, 1024], BF16, tag="osb")
                            for sub in range(2):
                                ec = eg * 2 + sub
                                p_o = psO_.tile([128, 512], F32, tag="po" if psO_ is not ps else "ps")
                                for hh in range(HPC):
                                    nc.tensor.matmul(
                                        p_o[:],
                                        at_tiles[hh][:, ss * 128:(ss + 1) * 128],
                                        ow_t[:, hh * H + ec * 512: hh * H + (ec + 1) * 512],
                                        start=(hh == 0), stop=(hh == HPC - 1),
                                    )
                                dst = o_sb[:, sub * 512:(sub + 1) * 512]
                                if (ss * 2 + eg) % 2 == 0:
                                    nc.vector.tensor_copy(dst, p_o[:])
                                else:
                                    nc.gpsimd.tensor_copy(dst, p_o[:])
                            nc.sync.dma_start(
                                outP[s_base + qb * CH + ss * 128:
                                     s_base + qb * CH + (ss + 1) * 128,
                                     eg * 1024:(eg + 1) * 1024],
                                o_sb[:],
                            )

                h_tiles = {}
                if b == 0:
                    h_tiles[0] = load_chunk(0, interleave=[
                        lambda: load_wqk_m(0, half=1),
                        lambda: load_wqk_m(1),
                        lambda: (load_wqk_m(2), load_wqk_m(3)),
                        lambda: (load_wv_half(0), load_wv_half(1)),
                    ])
                else:
                    h_tiles[0] = load_chunk(0)
                if causal:
                    # software pipeline: attention stage A (scores..at) for
                    # q-block n-1 is emitted with chunk n; its o_proj is
                    # delayed one more slot so the at-mult chain drains under
                    # chunk n+1's qkv matmuls.
                    pend = {}
                    for n in range(NCH):
                        do_qkv_chunk(n, h_tiles.pop(n))
                        do_rope_chunk(n)
                        if OPROJ_DELAY:
                            if n - 1 - LAG >= 0:
                                do_oproj(n - 1 - LAG, pend.pop(n - 1 - LAG))
                            if n >= LAG:
                                pend[n - LAG] = attention_stage_a(n - LAG)
                        elif n >= LAG:
                            do_oproj(n - LAG, attention_stage_a(n - LAG))
                    if OPROJ_DELAY:
                        for qb in range(NCH - LAG - 1, NCH):
                            if qb in pend:
                                do_oproj(qb, pend.pop(qb))
                            if qb + LAG < NCH:
                                pend[qb + LAG] = attention_stage_a(qb + LAG)
                    else:
                        for t in range(LAG):
                            do_oproj(NCH - LAG + t,
                                     attention_stage_a(NCH - LAG + t))
                else:
                    for n in range(NCH):
                        do_qkv_chunk(n, h_tiles.pop(n))
                    for n in range(NCH):
                        do_rope_chunk(n)
                    for qb in range(NCH):
                        do_oproj(qb, attention_stage_a(qb))
    return nc


def _adiag_pattern():
    p = np.arange(128)[:, None]
    j = np.arange(512)[None, :]
    out = np.zeros((128, 512), dtype=np.float32)
    out[:, 0:256] = np.where(p <= j[:, 0:256], 0.0, MASK_NEG)
    out[:, 256:384] = MASK_NEG
    out[:, 384:512] = np.where(p <= (j[:, 384:512] - 384), 0.0, MASK_NEG)
    return out


def _host_prep(hidden_states, position_ids, attention_mask, qkv_w, qkv_b, o_w):
    import ml_dtypes
    bf16 = ml_dtypes.bfloat16

    hidden_states = np.asarray(hidden_states, dtype=np.float32)
    position_ids = np.asarray(position_ids)
    attention_mask = np.asarray(attention_mask, dtype=np.float32)
    qkv_w = np.asarray(qkv_w, dtype=np.float32)
    qkv_b = np.asarray(qkv_b, dtype=np.float32)
    o_w = np.asarray(o_w, dtype=np.float32)

    # mask mode detection
    causal = np.triu(np.full((S, S), -1e9, dtype=np.float32), k=1)
    m = attention_mask.reshape(B, S, S)
    if all(np.array_equal(m[b], causal) for b in range(B)):
        mask_mode = "causal"
    elif not attention_mask.any():
        mask_mode = "dense"
    else:
        mask_mode = "generic"

    # rope tables
    half = HD // 2
    inv = (1.0 / ROPE_BASE ** (np.arange(half, dtype=np.float64) / half))
    freqs = position_ids.astype(np.float64).reshape(BS, 1) * inv[None, :]  # [BS,64]
    c = np.cos(freqs).T  # [64, BS]
    s_ = np.sin(freqs).T
    cosT = np.concatenate([c, c], 0).astype(bf16)
    sinS = np.concatenate([-s_, s_], 0).astype(bf16)

    hT = np.ascontiguousarray(hidden_states.reshape(BS, H).T).astype(bf16)

    perm = np.zeros((128, 128), dtype=np.float32)
    for dp in range(128):
        perm[(dp + 64) % 128, dp] = 1.0  # out[dp] = in[(dp+64)%128]

    shared = {
        "hT": hT, "cosT": cosT, "sinS": sinS,
        "permP": perm.astype(bf16),
        "ones128": np.ones((128, 128), dtype=bf16),
    }
    if mask_mode == "causal":
        shared["adiag"] = _adiag_pattern().astype(bf16)
    if mask_mode == "generic":
        shared["maskT"] = np.ascontiguousarray(
            np.transpose(m, (0, 2, 1)) / SCALE
        ).astype(np.float32)

    in_maps = []
    for c_id in range(8):
        r = c_id * DLOC
        wqk = np.vstack([qkv_w[r:r + DLOC], qkv_w[H + r:H + r + DLOC]])      # [512, H]
        wv = qkv_w[2 * H + r: 2 * H + r + DLOC]                               # [256, H]
        bqk = np.concatenate([qkv_b[r:r + DLOC], qkv_b[H + r:H + r + DLOC]])  # [512]
        im = dict(shared)
        # m-major packing: wqkP[p, m*KT*128 + kt*128 + c] = wqk[m*128+c, kt*128+p]
        wqkT = np.ascontiguousarray(wqk.T)                 # [H, 512]
        im["wqkP"] = np.ascontiguousarray(
            wqkT.reshape(KT, 128, 4, 128).transpose(1, 2, 0, 3).reshape(128, -1)
        ).astype(bf16)
        wvT = np.ascontiguousarray(wv.T)                   # [H, 256]
        im["wvP"] = np.ascontiguousarray(
            wvT.reshape(KT, 128, DLOC).transpose(1, 0, 2).reshape(128, -1)
        ).astype(bf16)
        owT = np.ascontiguousarray(o_w[:, r:r + DLOC].T)   # [256, H]
        im["owP"] = np.ascontiguousarray(
            owT.reshape(2, 128, H).transpose(1, 0, 2).reshape(128, -1)
        ).astype(bf16)
        im["bqkT"] = np.ascontiguousarray(bqk.reshape(4, 128).T)
        in_maps.append(im)
    post_bias = qkv_b[2 * H:3 * H] @ o_w.T  # [H], exact since sum(probs)=1
    return mask_mode, in_maps, post_bias


def kernel(**inputs) -> np.ndarray:
    import os
    import sys
    # The devices are reached through the axon PJRT proxy; make sure a
    # JAX_PLATFORMS=cpu pin (used for CPU-side reference runs) doesn't hide
    # them if jax hasn't been imported yet.
    if os.environ.get("JAX_PLATFORMS") == "cpu" and "jax" not in sys.modules:
        del os.environ["JAX_PLATFORMS"]
    from concourse.bass_utils import run_bass_kernel_spmd

    mask_mode, in_maps, post_bias = _host_prep(**inputs)
    nc = build_kernel(mask_mode)
    res = run_bass_kernel_spmd(nc, in_maps, core_ids=list(range(8)), trace=False)
    out = np.zeros((BS, H), dtype=np.float64)
    for r in res.results:
        out += r["outP"].astype(np.float64)
    out += post_bias.astype(np.float64)[None, :]
    return out.astype(np.float32).reshape(B, S, H)
